# revision 1
# baseline (speedup 1.0000x reference)
"""DGLJTMPN message-passing network on 8 Trainium2 NeuronCores (Bass/Tile).

Algorithm (mathematically identical to the reference):
  The loopy-BP line-graph aggregation  accum = segment_sum(msg[lg_src], lg_dst)
  is rewritten with node-level sums:  accum[e] = S[src[e]] - (backtracking
  partners), where S = segment_sum(msg, edge_dst).  The missing/backtracking
  pairs (the complement of the given lg list w.r.t. the full line graph) are
  folded into extra host-built one-hot "virtual columns", so each edge reads
  exactly one row of U = (S + node_alpha) @ W_h per iteration.

Sharding: nodes/edges/graphs are split into 8 contiguous graph-aligned
ranges; edges live on the core owning their dst node, so S/U shards are
disjoint.  Per iteration each core computes its U shard, an AllGather
replicates U, and a runtime-indexed dma_gather fetches U[src[e]] (two int16
half-table passes).  Scatter-adds are one-hot matmuls on the tensor engine.
"""

import numpy as np
import ml_dtypes

P = 128
SC = 4            # edge chunks per node tile
EDGE_CAP = SC * P
H = 256
GROUP = 1024      # edges per dma_gather call (8 chunks of 128; SWDGE ring caps a single gather near 2048 descriptors)
GPC = GROUP // P  # chunks per group = 16
N_CORES = 8
DEPTH = 4
GCAP = 2 * P      # max graphs per core

F32 = np.float32
BF16 = ml_dtypes.bfloat16


# ======================================================================
# Host preprocessing
# ======================================================================

def _full_line_graph_keys(src, dst, E, N):
    indeg = np.bincount(dst, minlength=N)
    idx_sorted = np.argsort(dst, kind="stable")
    ptr = np.concatenate([[0], np.cumsum(indeg)]).astype(np.int64)
    counts = indeg[src]
    total = int(counts.sum())
    lg_dst = np.repeat(np.arange(E, dtype=np.int64), counts)
    cum = np.cumsum(counts) - counts
    within = np.arange(total) - np.repeat(cum, counts)
    lg_src = idx_sorted[np.repeat(ptr[src], counts) + within]
    return lg_src * E + lg_dst


def _prep(inputs, n_cores=N_CORES):
    x_nodes = np.asarray(inputs["x_nodes"], F32)
    x_edges = np.asarray(inputs["x_edges"], F32)
    tree_m = np.asarray(inputs["tree_m"], F32)
    W_i = np.asarray(inputs["W_i"], F32)
    W_h = np.asarray(inputs["W_h"], F32)
    W_o = np.asarray(inputs["W_o"], F32)
    b_o = np.asarray(inputs["b_o"], F32)
    src = np.asarray(inputs["edge_src"], np.int64)
    dst = np.asarray(inputs["edge_dst"], np.int64)
    lg_src = np.asarray(inputs["lg_src"], np.int64)
    lg_dst = np.asarray(inputs["lg_dst"], np.int64)
    tgt_nodes = np.asarray(inputs["tgt_nodes"], np.int64)
    tree_eid = np.asarray(inputs["tree_eid"], np.int64)
    graph_ids = np.asarray(inputs["graph_ids"], np.int64)
    n_graphs = int(inputs["n_graphs"])

    N = x_nodes.shape[0]
    E = x_edges.shape[0]
    AF = x_nodes.shape[1]
    KF = AF + x_edges.shape[1]

    # corrections: full-line-graph pairs missing from the given lg list
    full_keys = _full_line_graph_keys(src, dst, E, N)
    given_keys = lg_src * E + lg_dst
    missing = np.setdiff1d(full_keys, given_keys)
    assert np.setdiff1d(given_keys, full_keys).size == 0
    miss_e1 = (missing // E).astype(np.int64)
    miss_e2 = (missing % E).astype(np.int64)
    assert np.all(dst[miss_e1] == src[miss_e2])
    order = np.argsort(miss_e2, kind="stable")
    miss_e1, miss_e2 = miss_e1[order], miss_e2[order]
    corr_e2, corr_start = np.unique(miss_e2, return_index=True)
    corr_partners = {}
    for i, e2 in enumerate(corr_e2):
        lo = corr_start[i]
        hi = corr_start[i + 1] if i + 1 < len(corr_e2) else len(miss_e2)
        corr_partners[int(e2)] = miss_e1[lo:hi]
    virt_nodes = src[corr_e2] if len(corr_e2) else np.array([], np.int64)
    vdemand = np.bincount(virt_nodes, minlength=N)
    corr_by_node = {}
    for e2 in corr_e2:
        corr_by_node.setdefault(int(src[e2]), []).append(int(e2))

    # graph-aligned node cuts
    gcnt = np.bincount(graph_ids, minlength=n_graphs)
    gcum = np.concatenate([[0], np.cumsum(gcnt)])
    cuts = [0]
    for c in range(1, n_cores):
        g = int(np.argmin(np.abs(gcum - c * N / n_cores)))
        cuts.append(int(gcum[g]))
    cuts.append(N)
    cuts = np.asarray(cuts, np.int64)
    assert np.all(np.diff(cuts) > 0)

    indeg = np.bincount(dst, minlength=N)
    assert indeg.max() <= EDGE_CAP
    edges_by_dst = np.argsort(dst, kind="stable")
    eptr = np.concatenate([[0], np.cumsum(indeg)]).astype(np.int64)
    tdeg = np.bincount(tgt_nodes, minlength=N)
    tpairs_by_tgt = np.argsort(tgt_nodes, kind="stable")
    tptr = np.concatenate([[0], np.cumsum(tdeg)]).astype(np.int64)

    # tile packing
    per_core_tiles = []
    for c in range(n_cores):
        nlo, nhi = int(cuts[c]), int(cuts[c + 1])
        tiles, cur, cur_slots, cur_edges = [], [], 0, 0
        for n in range(nlo, nhi):
            ns, ne = 1 + int(vdemand[n]), int(indeg[n])
            cap = P - 1 if c == 0 else P
            if cur and (cur_slots + ns > cap or cur_edges + ne > EDGE_CAP):
                tiles.append(cur)
                cur, cur_slots, cur_edges = [], 0, 0
            cur.append(n)
            cur_slots += ns
            cur_edges += ne
        if cur:
            tiles.append(cur)
        per_core_tiles.append(tiles)

    Kn = max(len(t) for t in per_core_tiles)
    Kn = -(-Kn // 4) * 4
    Kh = Kn // 2
    CORE_ROWS = P * Kn
    R = CORE_ROWS * n_cores
    RT = R // 2               # rows per split table (= one AG output)
    assert RT <= 32767
    E_slab = Kn * EDGE_CAP
    n_groups = E_slab // GROUP

    SCT = 1
    for c in range(n_cores):
        for tile in per_core_tiles[c]:
            SCT = max(SCT, -(-int(sum(tdeg[n] for n in tile)) // P))
    T_slab = Kn * SCT * P

    meta = dict(N=N, E=E, AF=AF, KF=KF, Kn=Kn, Kh=Kh, SCT=SCT,
                CORE_ROWS=CORE_ROWS, R=R, RT=RT, E_slab=E_slab,
                n_groups=n_groups, T_slab=T_slab, n_cores=n_cores,
                n_graphs=n_graphs, n_corr=len(corr_e2))

    HROWS = P * Kh            # rows per core per table
    # (tab, row) for node slot (c, t, j); zero rows live on core 0 whose
    # tiles are all capped at P-1 slots, so slot P-1 is always free there.
    def tab_row(c, t, j):
        if t < Kh:
            return 0, HROWS * c + P * t + j
        return 1, HROWS * c + P * (t - Kh) + j
    z_row = [P - 1, P - 1]    # core 0, tile 0 / tile Kh, slot 127

    # node slot assignment (global): tab*RT + row packed into one int
    node_row = np.full(N, -1, np.int64)
    virt_slot = {}
    node_tj = {}
    for c in range(n_cores):
        for t, tile in enumerate(per_core_tiles[c]):
            j = 0
            for n in tile:
                tb, rw = tab_row(c, t, j)
                node_row[n] = tb * RT + rw
                node_tj[n] = (t, j)
                j += 1
                for e2 in corr_by_node.get(n, []):
                    tb, rw = tab_row(c, t, j)
                    virt_slot[e2] = tb * RT + rw
                    j += 1
            assert j <= (P - 1 if c == 0 else P)
    assert np.all(node_row >= 0)
    edge_row = node_row[src].copy()
    for e2, row in virt_slot.items():
        edge_row[e2] = row

    glo_ghi = []
    per_core = []
    for c in range(n_cores):
        nlo, nhi = int(cuts[c]), int(cuts[c + 1])
        tiles = per_core_tiles[c]
        glo = int(graph_ids[nlo])
        ghi = int(graph_ids[nhi - 1]) + 1
        assert ghi - glo <= GCAP
        glo_ghi.append((glo, ghi))

        sel = np.zeros((Kn, SC, P, P), F32)
        seltree = np.zeros((Kn, SCT, P, P), F32)
        tree_slab = np.zeros((T_slab, H), F32)
        xe_catT = np.zeros((KF, E_slab), F32)
        xnodesT = np.zeros((AF + 1, CORE_ROWS), F32)
        xnodesT[AF, :] = 1.0
        poolw = np.zeros((Kn, P, GCAP), F32)
        idx_rows = np.full(E_slab, -1, np.int64)

        for t, tile in enumerate(tiles):
            pos_of_edge = {}
            k = 0
            for n in tile:
                j = node_tj[n][1]
                xnodesT[:AF, P * t + j] = x_nodes[n]
                g = int(graph_ids[n])
                poolw[t, j, g - glo] = 1.0 / max(int(gcnt[g]), 1)
                for e in edges_by_dst[eptr[n]:eptr[n + 1]]:
                    slab_pos = EDGE_CAP * t + k
                    pos_of_edge[int(e)] = k
                    sel[t, k // P, k % P, j] = 1.0
                    idx_rows[slab_pos] = edge_row[e]
                    xe_catT[:AF, slab_pos] = x_nodes[src[e]]
                    xe_catT[AF:, slab_pos] = x_edges[e]
                    k += 1
            assert k <= EDGE_CAP
            # virtual columns
            for n in tile:
                jn = node_tj[n][1]
                for vi, e2 in enumerate(corr_by_node.get(n, [])):
                    jv = jn + 1 + vi
                    partners = set(corr_partners[e2].tolist())
                    for e in edges_by_dst[eptr[n]:eptr[n + 1]]:
                        if int(e) in partners:
                            continue
                        kk = pos_of_edge[int(e)]
                        sel[t, kk // P, kk % P, jv] = 1.0
            # tree pairs
            kt = 0
            for n in tile:
                j = node_tj[n][1]
                nvirt = len(corr_by_node.get(n, []))
                for pidx in tpairs_by_tgt[tptr[n]:tptr[n + 1]]:
                    tree_slab[SCT * P * t + kt] = tree_m[tree_eid[pidx]]
                    seltree[t, kt // P, kt % P, j] = 1.0
                    for vi in range(nvirt):
                        seltree[t, kt // P, kt % P, j + 1 + vi] = 1.0
                    kt += 1
            assert kt <= SCT * P

        in0 = (idx_rows >= 0) & (idx_rows < RT)
        in1 = idx_rows >= RT
        idx0 = np.where(in0, idx_rows, z_row[0])
        idx1 = np.where(in1, idx_rows - RT, z_row[1])
        assert 0 <= idx0.min() and idx0.max() < RT
        assert 0 <= idx1.min() and idx1.max() < RT

        def wrap(idx):
            # -> [128, n_groups, GROUP//16] with j = col*16 + (p % 16)
            w = idx.reshape(n_groups, GROUP // 16, 16)   # [g, col, p16]
            w = np.transpose(w, (2, 0, 1))               # [p16, g, col]
            w = np.tile(w, (P // 16, 1, 1))
            return np.ascontiguousarray(w.astype(np.int16))

        # sel regrouped for gather-group-major DMA:
        # [n_groups, 128(e), GPC, 128(j)]
        selg = np.transpose(
            sel.reshape(n_groups, GPC, P, P), (0, 2, 1, 3))
        per_core.append(dict(
            xe_catT=xe_catT.astype(BF16),
            sel=np.ascontiguousarray(selg.astype(BF16)),
            seltree=seltree,
            tree_slab=tree_slab,
            xnodesT=xnodesT.astype(BF16),
            poolw=poolw,
            idx0=wrap(idx0),
            idx1=wrap(idx1),
            wi=W_i.astype(BF16),
            wh=W_h.astype(BF16),
            wo1=np.ascontiguousarray(
                np.concatenate([W_o[:AF], b_o[None, :]], 0).astype(BF16)),
            wo2=np.ascontiguousarray(W_o[AF:].astype(BF16)),
        ))

    return per_core, meta, glo_ghi


# ======================================================================
# Bass program
# ======================================================================

def _build(meta):
    import os
    abl = set(os.environ.get("KERNEL_ABL", "").split(","))
    import concourse.bacc as bacc
    import concourse.tile as tile
    from concourse import mybir

    Kn, Kh, SCT = meta["Kn"], meta["Kh"], meta["SCT"]
    CORE_ROWS, R, RT = meta["CORE_ROWS"], meta["R"], meta["RT"]
    E_slab, n_groups, T_slab = meta["E_slab"], meta["n_groups"], meta["T_slab"]
    KF, AF = meta["KF"], meta["AF"]
    TPG = GPC // SC            # node tiles per gather group
    HROWS = P * Kh

    f32, bf16, i16 = mybir.dt.float32, mybir.dt.bfloat16, mybir.dt.int16
    RELU = mybir.ActivationFunctionType.Relu
    ADD = mybir.AluOpType.add

    nc = bacc.Bacc("TRN2", target_bir_lowering=False, num_devices=N_CORES)

    # kernel I/O
    xe_in = nc.dram_tensor("xe_catT", [KF, E_slab], bf16, kind="ExternalInput")
    sel_in = nc.dram_tensor("sel", [n_groups, P, GPC, P], bf16,
                            kind="ExternalInput")
    seltree_in = nc.dram_tensor("seltree", [Kn, SCT, P, P], f32,
                                kind="ExternalInput")
    tree_in = nc.dram_tensor("tree_slab", [T_slab, H], f32,
                             kind="ExternalInput")
    xn_in = nc.dram_tensor("xnodesT", [AF + 1, CORE_ROWS], bf16,
                           kind="ExternalInput")
    poolw_in = nc.dram_tensor("poolw", [Kn, P, GCAP], f32,
                              kind="ExternalInput")
    idx0_in = nc.dram_tensor("idx0", [P, n_groups, GROUP // 16], i16,
                             kind="ExternalInput")
    idx1_in = nc.dram_tensor("idx1", [P, n_groups, GROUP // 16], i16,
                             kind="ExternalInput")
    wi_in = nc.dram_tensor("wi", [KF, H], bf16, kind="ExternalInput")
    wh_in = nc.dram_tensor("wh", [H, H], bf16, kind="ExternalInput")
    wo1_in = nc.dram_tensor("wo1", [AF + 1, H], bf16, kind="ExternalInput")
    wo2_in = nc.dram_tensor("wo2", [H, H], bf16, kind="ExternalInput")
    out_t = nc.dram_tensor("out", [GCAP, H], f32, kind="ExternalOutput")

    # internal DRAM
    msgin_hbm = nc.dram_tensor("msgin_hbm", [n_groups, P, GPC, H], bf16)
    ag_in = [nc.dram_tensor(f"ag_in_{h}", [HROWS, H], bf16) for h in range(2)]
    U_tab = [[nc.dram_tensor(f"U_{h}{i}", [RT, H], bf16, addr_space="Shared")
              for h in range(2)] for i in range(2)]

    with tile.TileContext(nc) as tc:
        with tc.tile_pool(name="const", bufs=1) as cpool, \
             tc.tile_pool(name="work", bufs=2) as wp, \
             tc.tile_pool(name="small", bufs=3) as sp, \
             tc.tile_pool(name="psum", bufs=2, space="PSUM") as pp, \
             tc.tile_pool(name="psum_acc", bufs=1, space="PSUM") as pacc:

            # ---- resident constants ----
            wi_t = cpool.tile([KF, H], bf16)
            nc.sync.dma_start(out=wi_t[:], in_=wi_in[:])
            wh0 = cpool.tile([P, H], bf16)
            wh1 = cpool.tile([P, H], bf16)
            nc.sync.dma_start(out=wh0[:], in_=wh_in[0:P, :])
            nc.sync.dma_start(out=wh1[:], in_=wh_in[P:H, :])
            wo1_t = cpool.tile([AF + 1, H], bf16)
            nc.sync.dma_start(out=wo1_t[:], in_=wo1_in[:])
            wo2_0 = cpool.tile([P, H], bf16)
            wo2_1 = cpool.tile([P, H], bf16)
            nc.sync.dma_start(out=wo2_0[:], in_=wo2_in[0:P, :])
            nc.sync.dma_start(out=wo2_1[:], in_=wo2_in[P:H, :])
            na_all = cpool.tile([P, Kn, 2, P], f32)
            idx0_t = cpool.tile([P, n_groups, GROUP // 16], i16)
            idx1_t = cpool.tile([P, n_groups, GROUP // 16], i16)
            nc.sync.dma_start(out=idx0_t[:], in_=idx0_in[:])
            nc.sync.dma_start(out=idx1_t[:], in_=idx1_in[:])

            # ---- stage A: node_alpha^T ----
            for t in range(Kn):
                ps_na = [pp.tile([P, P], f32, tag=f"ps_s{i}", name=f"ps_na{i}")
                         for i in range(2)]
                for ct in range(SCT):
                    tr = sp.tile([P, H], f32, tag="tr")
                    nc.sync.dma_start(
                        out=tr[:],
                        in_=tree_in[(t * SCT + ct) * P:(t * SCT + ct + 1) * P, :])
                    st = sp.tile([P, P], f32, tag="st")
                    nc.sync.dma_start(out=st[:], in_=seltree_in[t, ct])
                    for s in range(2):
                        nc.tensor.matmul(
                            out=ps_na[s][:],
                            lhsT=tr[:, s * P:(s + 1) * P], rhs=st[:],
                            start=(ct == 0), stop=(ct == SCT - 1))
                for s in range(2):
                    nc.vector.tensor_copy(out=na_all[:, t, s, :],
                                          in_=ps_na[s][:])

            # ---- sweeps ----
            for sw in range(DEPTH):
                last = sw == DEPTH - 1
                if last:
                    psG = [pacc.tile([P, H], f32, tag=f"psG{i}", name=f"psG{i}")
                           for i in range(2)]
                for grp in range(n_groups):
                    # message pre-activation for this group
                    if sw == 0:
                        mi = wp.tile([P, GPC, H], bf16, tag="mi")
                        xe_g = wp.tile([KF, GROUP], bf16, tag="xe")
                        nc.sync.dma_start(
                            out=xe_g[:],
                            in_=xe_in[:, grp * GROUP:(grp + 1) * GROUP])
                        for k in range(GPC):
                            ps_mi = pp.tile([P, H], f32, tag="ps_big", name="ps_mi")
                            nc.tensor.matmul(
                                out=ps_mi[:],
                                lhsT=xe_g[:, k * P:(k + 1) * P],
                                rhs=wi_t[:], start=True, stop=True)
                            nc.vector.tensor_copy(out=mi[:, k, :], in_=ps_mi[:])
                        nc.sync.dma_start(out=msgin_hbm[grp], in_=mi[:])
                        pre = mi
                    else:
                        mi = wp.tile([P, GPC, H], bf16, tag="mi")
                        nc.sync.dma_start(out=mi[:], in_=msgin_hbm[grp])
                        if "nogather" not in abl:
                            g0 = wp.tile([P, GPC, H], bf16, tag="g0")
                            g1 = wp.tile([P, GPC, H], bf16, tag="g1")
                            Up = U_tab[(sw + 1) % 2]
                            nc.gpsimd.dma_gather(
                                out_ap=g0[:], in_ap=Up[0][:],
                                idxs_ap=idx0_t[:, grp, :],
                                num_idxs=GROUP, num_idxs_reg=GROUP,
                                elem_size=H, queue_num=0)
                            nc.gpsimd.dma_gather(
                                out_ap=g1[:], in_ap=Up[1][:],
                                idxs_ap=idx1_t[:, grp, :],
                                num_idxs=GROUP, num_idxs_reg=GROUP,
                                elem_size=H, queue_num=0)
                            f0 = mi[:].rearrange("p a b -> p (a b)")
                            nc.vector.tensor_tensor(
                                out=f0, in0=f0,
                                in1=g0[:].rearrange("p a b -> p (a b)"), op=ADD)
                            nc.vector.tensor_tensor(
                                out=f0, in0=f0,
                                in1=g1[:].rearrange("p a b -> p (a b)"), op=ADD)
                        pre = mi
                    msg = wp.tile([P, GPC, H], bf16, tag="msg")
                    nc.scalar.activation(
                        out=msg[:].rearrange("p a b -> p (a b)"),
                        in_=pre[:].rearrange("p a b -> p (a b)"), func=RELU)
                    selg = wp.tile([P, GPC, P], bf16, tag="selg")
                    nc.sync.dma_start(out=selg[:], in_=sel_in[grp])
                    for tt in range(TPG):
                        t = grp * TPG + tt
                        psS = [pp.tile([P, P], f32, tag=f"ps_s{i}", name=f"psS{i}")
                               for i in range(2)]
                        for cc in range(SC):
                            if "nosmm" in abl:
                                break
                            k = tt * SC + cc
                            for s in range(2):
                                nc.tensor.matmul(
                                    out=psS[s][:],
                                    lhsT=msg[:, k, s * P:(s + 1) * P],
                                    rhs=selg[:, k, :],
                                    start=(cc == 0), stop=(cc == SC - 1))
                        TT = [sp.tile([P, P], bf16, tag=f"TT{i}", name=f"TT{i}")
                              for i in range(2)]
                        for s in range(2):
                            if "nosmm" in abl:
                                nc.vector.tensor_copy(out=TT[s][:],
                                                      in_=na_all[:, t, s, :])
                            else:
                                nc.vector.tensor_tensor(
                                    out=TT[s][:], in0=psS[s][:],
                                    in1=na_all[:, t, s, :], op=ADD)
                        if not last:
                            u_sb = sp.tile([P, H], bf16, tag="u_sb")
                            if "noumm" in abl:
                                nc.vector.tensor_copy(
                                    out=u_sb[:, 0:P],  in_=TT[0][:])
                                nc.vector.tensor_copy(
                                    out=u_sb[:, P:H], in_=TT[1][:])
                            else:
                                psU = pp.tile([P, H], f32, tag="ps_big",
                                              name="psU")
                                nc.tensor.matmul(out=psU[:], lhsT=TT[0][:],
                                                 rhs=wh0[:], start=True,
                                                 stop=False)
                                nc.tensor.matmul(out=psU[:], lhsT=TT[1][:],
                                                 rhs=wh1[:], start=False,
                                                 stop=True)
                                nc.vector.tensor_copy(out=u_sb[:], in_=psU[:])
                            agt = ag_in[0] if t < Kh else ag_in[1]
                            tl = t if t < Kh else t - Kh
                            nc.sync.dma_start(
                                out=agt[tl * P:(tl + 1) * P, :], in_=u_sb[:])
                            if t == Kh - 1 and "noag" not in abl:
                                nc.gpsimd.collective_compute(
                                    "AllGather", mybir.AluOpType.bypass,
                                    replica_groups=[list(range(N_CORES))],
                                    ins=[ag_in[0][:].opt()],
                                    outs=[U_tab[sw % 2][0][:].opt()])
                        else:
                            xn_t = sp.tile([AF + 1, P], bf16, tag="xn_t")
                            nc.sync.dma_start(
                                out=xn_t[:],
                                in_=xn_in[:, t * P:(t + 1) * P])
                            psH = pp.tile([P, H], f32, tag="ps_big", name="psH")
                            nc.tensor.matmul(out=psH[:], lhsT=xn_t[:],
                                             rhs=wo1_t[:], start=True,
                                             stop=False)
                            nc.tensor.matmul(out=psH[:], lhsT=TT[0][:],
                                             rhs=wo2_0[:], start=False,
                                             stop=False)
                            nc.tensor.matmul(out=psH[:], lhsT=TT[1][:],
                                             rhs=wo2_1[:], start=False,
                                             stop=True)
                            h_sb = sp.tile([P, H], f32, tag="h_sb")
                            nc.scalar.activation(out=h_sb[:], in_=psH[:],
                                                 func=RELU)
                            pw_t = sp.tile([P, GCAP], f32, tag="pw_t")
                            nc.sync.dma_start(out=pw_t[:], in_=poolw_in[t])
                            for s in range(2):
                                nc.tensor.matmul(
                                    out=psG[s][:],
                                    lhsT=pw_t[:, s * P:(s + 1) * P],
                                    rhs=h_sb[:],
                                    start=(t == 0), stop=(t == Kn - 1))
                if not last:
                    if "noag" not in abl:
                        nc.gpsimd.collective_compute(
                            "AllGather", mybir.AluOpType.bypass,
                            replica_groups=[list(range(N_CORES))],
                            ins=[ag_in[1][:].opt()],
                            outs=[U_tab[sw % 2][1][:].opt()])
                else:
                    for s in range(2):
                        og = sp.tile([P, H], f32, tag="og")
                        nc.vector.tensor_copy(out=og[:], in_=psG[s][:])
                        nc.sync.dma_start(
                            out=out_t[s * P:(s + 1) * P, :], in_=og[:])

    nc.compile()
    return nc


# ======================================================================
# Entry point
# ======================================================================

_last_results = None


def kernel(**inputs):
    from concourse.bass_utils import run_bass_kernel_spmd

    per_core, meta, glo_ghi = _prep(inputs)
    nc = _build(meta)
    in_maps = [{k: v for k, v in pc.items()} for pc in per_core]
    res = run_bass_kernel_spmd(nc, in_maps, core_ids=list(range(N_CORES)))
    global _last_results
    _last_results = res

    G = meta["n_graphs"]
    out = np.zeros((G, H), F32)
    for c in range(N_CORES):
        glo, ghi = glo_ghi[c]
        out[glo:ghi] = res.results[c]["out"][: ghi - glo]
    return out



# revision 5
# speedup vs baseline: 1.6627x; 1.6627x over previous
"""DGLJTMPN message-passing network on 8 Trainium2 NeuronCores (Bass/Tile).

Algorithm (mathematically identical to the reference):
  The loopy-BP line-graph aggregation  accum = segment_sum(msg[lg_src], lg_dst)
  is rewritten with node-level sums:  accum[e] = S[src[e]] - (backtracking
  partners), where S = segment_sum(msg, edge_dst).  The missing/backtracking
  pairs (the complement of the given lg list w.r.t. the full line graph) are
  folded into extra host-built one-hot "virtual columns", so each edge reads
  exactly one row of U = (S + node_alpha) @ W_h per iteration.

Sharding: nodes/edges/graphs are split into 8 contiguous graph-aligned
ranges; edges live on the core owning their dst node, so S/U shards are
disjoint.  Per iteration each core computes its U shard, an AllGather
replicates U (in two int16-addressable half tables), and a runtime-indexed
dma_gather fetches U[src[e]].  Each tile's edge slab is split into two
256-slot class regions by src half-table, so every edge is gathered exactly
once (no zero-row double gathers).  msg_input stays resident in SBUF across
all sweeps.  Scatter-adds are one-hot matmuls on the tensor engine.
"""

import numpy as np
import ml_dtypes

P = 128
SC = 4            # edge chunks per node tile
EDGE_CAP = SC * P
CLASS_CAP = EDGE_CAP // 2   # per-tile cap on edges per src-table class
H = 256
N_CORES = 8
DEPTH = 4
GCAP = 2 * P      # max graphs per core

F32 = np.float32
BF16 = ml_dtypes.bfloat16


# ======================================================================
# Host preprocessing
# ======================================================================

def _full_line_graph_keys(src, dst, E, N):
    indeg = np.bincount(dst, minlength=N)
    idx_sorted = np.argsort(dst, kind="stable")
    ptr = np.concatenate([[0], np.cumsum(indeg)]).astype(np.int64)
    counts = indeg[src]
    total = int(counts.sum())
    lg_dst = np.repeat(np.arange(E, dtype=np.int64), counts)
    cum = np.cumsum(counts) - counts
    within = np.arange(total) - np.repeat(cum, counts)
    lg_src = idx_sorted[np.repeat(ptr[src], counts) + within]
    return lg_src * E + lg_dst


def _prep(inputs, n_cores=N_CORES):
    x_nodes = np.asarray(inputs["x_nodes"], F32)
    x_edges = np.asarray(inputs["x_edges"], F32)
    tree_m = np.asarray(inputs["tree_m"], F32)
    W_i = np.asarray(inputs["W_i"], F32)
    W_h = np.asarray(inputs["W_h"], F32)
    W_o = np.asarray(inputs["W_o"], F32)
    b_o = np.asarray(inputs["b_o"], F32)
    src = np.asarray(inputs["edge_src"], np.int64)
    dst = np.asarray(inputs["edge_dst"], np.int64)
    lg_src = np.asarray(inputs["lg_src"], np.int64)
    lg_dst = np.asarray(inputs["lg_dst"], np.int64)
    tgt_nodes = np.asarray(inputs["tgt_nodes"], np.int64)
    tree_eid = np.asarray(inputs["tree_eid"], np.int64)
    graph_ids = np.asarray(inputs["graph_ids"], np.int64)
    n_graphs = int(inputs["n_graphs"])

    N = x_nodes.shape[0]
    E = x_edges.shape[0]
    AF = x_nodes.shape[1]
    KF = AF + x_edges.shape[1]

    # corrections: full-line-graph pairs missing from the given lg list
    full_keys = _full_line_graph_keys(src, dst, E, N)
    given_keys = lg_src * E + lg_dst
    missing = np.setdiff1d(full_keys, given_keys)
    assert np.setdiff1d(given_keys, full_keys).size == 0
    miss_e1 = (missing // E).astype(np.int64)
    miss_e2 = (missing % E).astype(np.int64)
    assert np.all(dst[miss_e1] == src[miss_e2])
    order = np.argsort(miss_e2, kind="stable")
    miss_e1, miss_e2 = miss_e1[order], miss_e2[order]
    corr_e2, corr_start = np.unique(miss_e2, return_index=True)
    corr_partners = {}
    for i, e2 in enumerate(corr_e2):
        lo = corr_start[i]
        hi = corr_start[i + 1] if i + 1 < len(corr_e2) else len(miss_e2)
        corr_partners[int(e2)] = miss_e1[lo:hi]
    virt_nodes = src[corr_e2] if len(corr_e2) else np.array([], np.int64)
    vdemand = np.bincount(virt_nodes, minlength=N)
    corr_by_node = {}
    for e2 in corr_e2:
        corr_by_node.setdefault(int(src[e2]), []).append(int(e2))

    # graph-aligned node cuts
    gcnt = np.bincount(graph_ids, minlength=n_graphs)
    gcum = np.concatenate([[0], np.cumsum(gcnt)])
    cuts = [0]
    for c in range(1, n_cores):
        g = int(np.argmin(np.abs(gcum - c * N / n_cores)))
        cuts.append(int(gcum[g]))
    cuts.append(N)
    cuts = np.asarray(cuts, np.int64)
    assert np.all(np.diff(cuts) > 0)

    indeg = np.bincount(dst, minlength=N)
    assert indeg.max() <= CLASS_CAP
    edges_by_dst = np.argsort(dst, kind="stable")
    eptr = np.concatenate([[0], np.cumsum(indeg)]).astype(np.int64)
    tdeg = np.bincount(tgt_nodes, minlength=N)
    tpairs_by_tgt = np.argsort(tgt_nodes, kind="stable")
    tptr = np.concatenate([[0], np.cumsum(tdeg)]).astype(np.int64)

    # ---- tile packing with per-class edge caps ----
    # class of edge e = half-table of src[e]'s U row, which depends on the
    # packing itself (tile index < Kh -> table 0).  Pack with an estimated
    # class map and a safety margin on the per-class caps, then accept any
    # packing whose TRUE per-tile class counts fit within CLASS_CAP.
    # Iteration 0 packs without class caps to seed the estimates.
    ecls = np.zeros(E, np.int8)
    margin = 16
    for _it in range(12):
        class_capped = _it > 0
        ccap = CLASS_CAP - margin
        per_core_tiles = []
        for c in range(n_cores):
            nlo, nhi = int(cuts[c]), int(cuts[c + 1])
            tiles, cur, cur_slots = [], [], 0
            cur_c = [0, 0]
            cap = P - 1 if c == 0 else P
            for n in range(nlo, nhi):
                ns = 1 + int(vdemand[n])
                ee = edges_by_dst[eptr[n]:eptr[n + 1]]
                n0 = int(np.count_nonzero(ecls[ee] == 0))
                n1 = len(ee) - n0
                if class_capped:
                    full = (cur_c[0] + n0 > ccap or cur_c[1] + n1 > ccap)
                else:
                    full = cur_c[0] + cur_c[1] + n0 + n1 > EDGE_CAP
                if cur and (cur_slots + ns > cap or full):
                    tiles.append(cur)
                    cur, cur_slots, cur_c = [], 0, [0, 0]
                cur.append(n)
                cur_slots += ns
                cur_c[0] += n0
                cur_c[1] += n1
            if cur:
                tiles.append(cur)
            per_core_tiles.append(tiles)

        Kn = max(len(t) for t in per_core_tiles)
        Kn = -(-Kn // 4) * 4
        Kh = Kn // 2
        CORE_ROWS = P * Kn
        R = CORE_ROWS * n_cores
        RT = R // 2
        HROWS = P * Kh

        def tab_row(c, t, j):
            if t < Kh:
                return 0, HROWS * c + P * t + j
            return 1, HROWS * c + P * (t - Kh) + j

        node_row = np.full(N, -1, np.int64)
        virt_slot = {}
        node_tj = {}
        for c in range(n_cores):
            for t, tile in enumerate(per_core_tiles[c]):
                j = 0
                for n in tile:
                    tb, rw = tab_row(c, t, j)
                    node_row[n] = tb * RT + rw
                    node_tj[n] = (t, j)
                    j += 1
                    for e2 in corr_by_node.get(n, []):
                        tb, rw = tab_row(c, t, j)
                        virt_slot[e2] = tb * RT + rw
                        j += 1
                assert j <= (P - 1 if c == 0 else P)
        assert np.all(node_row >= 0)
        edge_row = node_row[src].copy()
        for e2, row in virt_slot.items():
            edge_row[e2] = row

        true_cls = (edge_row >= RT).astype(np.int8)
        if class_capped:
            ok = True
            for c in range(n_cores):
                for tile in per_core_tiles[c]:
                    tc = [0, 0]
                    for n in tile:
                        for e in edges_by_dst[eptr[n]:eptr[n + 1]]:
                            tc[true_cls[e]] += 1
                    if tc[0] > CLASS_CAP or tc[1] > CLASS_CAP:
                        ok = False
            if ok:
                ecls = true_cls
                break
            margin = min(margin + 16, 96)
        ecls = true_cls
    else:
        raise RuntimeError("edge-class packing did not converge")
    assert RT <= 32767, RT

    n_groups = Kh               # 2 tiles per group
    E_slab = Kn * EDGE_CAP
    GROUP = 2 * EDGE_CAP        # slots per group

    SCT = 1
    for c in range(n_cores):
        for tile in per_core_tiles[c]:
            SCT = max(SCT, -(-int(sum(tdeg[n] for n in tile)) // P))
    T_slab = Kn * SCT * P

    meta = dict(N=N, E=E, AF=AF, KF=KF, Kn=Kn, Kh=Kh, SCT=SCT,
                CORE_ROWS=CORE_ROWS, R=R, RT=RT, E_slab=E_slab,
                n_groups=n_groups, T_slab=T_slab, n_cores=n_cores,
                n_graphs=n_graphs, n_corr=len(corr_e2))

    # zero rows (core 0 caps slots at P-1, so slot P-1 of tile 0 / tile Kh
    # is always free there): local row 127 in each table.
    z_local = [P - 1, P - 1]

    def wrap(idx):
        # idx [512] -> [128, 32] with idx j at (partition j%16, col j//16)
        w = idx.reshape(-1, 16)            # [col, lane]
        w = np.ascontiguousarray(w.T)      # [16, col]
        w = np.tile(w, (P // 16, 1))
        return w.astype(np.int16)

    glo_ghi = []
    per_core = []
    for c in range(n_cores):
        nlo, nhi = int(cuts[c]), int(cuts[c + 1])
        tiles = per_core_tiles[c]
        glo = int(graph_ids[nlo])
        ghi = int(graph_ids[nhi - 1]) + 1
        assert ghi - glo <= GCAP
        glo_ghi.append((glo, ghi))

        sel = np.zeros((n_groups, 8, P, P), F32)
        seltree = np.zeros((Kn, SCT, P, P), F32)
        tree_slab = np.zeros((T_slab, H), F32)
        xe_catT = np.zeros((KF, E_slab), F32)
        xnodesT = np.zeros((AF + 1, CORE_ROWS), F32)
        xnodesT[AF, :] = 1.0
        poolw = np.zeros((Kn, P, GCAP), F32)
        idx_all = np.zeros((n_groups, 2, P, GROUP // 2 // 16), np.int16)

        for t, tile in enumerate(tiles):
            g, half = t // 2, t % 2
            # per-class edge placement: class s region of tile t =
            # group chunks {4s + 2*half, 4s + 2*half + 1}
            kcnt = [0, 0]           # edges placed per class
            pos_of_edge = {}
            rows_cls = [[], []]
            for n in tile:
                j = node_tj[n][1]
                xnodesT[:AF, P * t + j] = x_nodes[n]
                gg = int(graph_ids[n])
                poolw[t, j, gg - glo] = 1.0 / max(int(gcnt[gg]), 1)
                for e in edges_by_dst[eptr[n]:eptr[n + 1]]:
                    s = int(ecls[e])
                    k_in = kcnt[s]
                    kcnt[s] += 1
                    chunk = 4 * s + 2 * half + k_in // P
                    i = k_in % P
                    slab_pos = (8 * g + chunk) * P + i
                    pos_of_edge[int(e)] = (chunk, i)
                    sel[g, chunk, i, j] = 1.0
                    row = int(edge_row[e]) - s * RT
                    rows_cls[s].append(row)
                    xe_catT[:AF, slab_pos] = x_nodes[src[e]]
                    xe_catT[AF:, slab_pos] = x_edges[e]
            assert kcnt[0] <= CLASS_CAP and kcnt[1] <= CLASS_CAP
            for s in range(2):
                lst = rows_cls[s] + [z_local[s]] * (CLASS_CAP - kcnt[s])
                arr = np.asarray(lst, np.int64)
                assert arr.min() >= 0 and arr.max() < RT
                # tile half occupies positions half*CLASS_CAP .. within the
                # group's class-s gather (512 idxs spanning both tiles)
                idx_all[g, s, :, half * (CLASS_CAP // 16):
                       (half + 1) * (CLASS_CAP // 16)] = wrap(arr)
            # virtual columns
            for n in tile:
                jn = node_tj[n][1]
                for vi, e2 in enumerate(corr_by_node.get(n, [])):
                    jv = jn + 1 + vi
                    partners = set(corr_partners[e2].tolist())
                    for e in edges_by_dst[eptr[n]:eptr[n + 1]]:
                        if int(e) in partners:
                            continue
                        ck, ii = pos_of_edge[int(e)]
                        sel[g, ck, ii, jv] = 1.0
            # tree pairs
            kt = 0
            for n in tile:
                j = node_tj[n][1]
                nvirt = len(corr_by_node.get(n, []))
                for pidx in tpairs_by_tgt[tptr[n]:tptr[n + 1]]:
                    tree_slab[SCT * P * t + kt] = tree_m[tree_eid[pidx]]
                    seltree[t, kt // P, kt % P, j] = 1.0
                    for vi in range(nvirt):
                        seltree[t, kt // P, kt % P, j + 1 + vi] = 1.0
                    kt += 1
            assert kt <= SCT * P

        # sel regrouped for DMA: [n_groups, 128(edge), 8, 128(slot)]
        selg = np.ascontiguousarray(np.transpose(sel, (0, 2, 1, 3)))
        ident = np.eye(P, dtype=F32)
        per_core.append(dict(
            xe_catT=xe_catT.astype(BF16),
            sel=selg.astype(BF16),
            seltree=seltree,
            tree_slab=tree_slab,
            xnodesT=xnodesT.astype(BF16),
            poolw=poolw,
            idxs=np.ascontiguousarray(
                np.transpose(idx_all, (2, 0, 1, 3))),   # [128, g, 2, 32]
            ident=ident.astype(BF16),
            wi=W_i.astype(BF16),
            wh=W_h.astype(BF16),
            wo1=np.ascontiguousarray(
                np.concatenate([W_o[:AF], b_o[None, :]], 0).astype(BF16)),
            wo2=np.ascontiguousarray(W_o[AF:].astype(BF16)),
        ))

    return per_core, meta, glo_ghi


# ======================================================================
# Bass program
# ======================================================================

def _build(meta):
    import os
    abl = set(os.environ.get("KERNEL_ABL", "").split(","))
    import concourse.bacc as bacc
    import concourse.tile as tile
    from concourse import mybir

    Kn, Kh, SCT = meta["Kn"], meta["Kh"], meta["SCT"]
    CORE_ROWS, R, RT = meta["CORE_ROWS"], meta["R"], meta["RT"]
    E_slab, n_groups, T_slab = meta["E_slab"], meta["n_groups"], meta["T_slab"]
    KF, AF = meta["KF"], meta["AF"]
    HROWS = P * Kh
    NCH = E_slab // P           # total slab chunks ( = 8 * n_groups )
    IW = CLASS_CAP * 2 // 16    # idx cols per (group, class) = 32

    f32, bf16, i16 = mybir.dt.float32, mybir.dt.bfloat16, mybir.dt.int16
    RELU = mybir.ActivationFunctionType.Relu
    ADD = mybir.AluOpType.add

    nc = bacc.Bacc("TRN2", target_bir_lowering=False, num_devices=N_CORES)

    # kernel I/O
    xe_in = nc.dram_tensor("xe_catT", [KF, E_slab], bf16, kind="ExternalInput")
    sel_in = nc.dram_tensor("sel", [n_groups, P, 8, P], bf16,
                            kind="ExternalInput")
    seltree_in = nc.dram_tensor("seltree", [Kn, SCT, P, P], f32,
                                kind="ExternalInput")
    tree_in = nc.dram_tensor("tree_slab", [T_slab, H], f32,
                             kind="ExternalInput")
    xn_in = nc.dram_tensor("xnodesT", [AF + 1, CORE_ROWS], bf16,
                           kind="ExternalInput")
    poolw_in = nc.dram_tensor("poolw", [Kn, P, GCAP], f32,
                              kind="ExternalInput")
    idx_in = nc.dram_tensor("idxs", [P, n_groups, 2, IW], i16,
                            kind="ExternalInput")
    id_in = nc.dram_tensor("ident", [P, P], bf16, kind="ExternalInput")
    wi_in = nc.dram_tensor("wi", [KF, H], bf16, kind="ExternalInput")
    wh_in = nc.dram_tensor("wh", [H, H], bf16, kind="ExternalInput")
    wo1_in = nc.dram_tensor("wo1", [AF + 1, H], bf16, kind="ExternalInput")
    wo2_in = nc.dram_tensor("wo2", [H, H], bf16, kind="ExternalInput")
    out_t = nc.dram_tensor("out", [GCAP, H], f32, kind="ExternalOutput")

    # internal DRAM
    ag_in = [nc.dram_tensor(f"ag_in_{h}", [HROWS, H], bf16) for h in range(2)]
    U_tab = [[nc.dram_tensor(f"U_{h}{i}", [RT, H], bf16, addr_space="Shared")
              for h in range(2)] for i in range(2)]

    with tile.TileContext(nc) as tc:
        with tc.tile_pool(name="const", bufs=1) as cpool, \
             tc.tile_pool(name="work", bufs=2) as wp, \
             tc.tile_pool(name="small", bufs=3) as sp, \
             tc.tile_pool(name="psum", bufs=2, space="PSUM") as pp, \
             tc.tile_pool(name="psum_acc", bufs=1, space="PSUM") as pacc:

            # ---- resident constants ----
            wi_t = cpool.tile([KF, H], bf16)
            nc.sync.dma_start(out=wi_t[:], in_=wi_in[:])
            wh0 = cpool.tile([P, H], bf16)
            wh1 = cpool.tile([P, H], bf16)
            nc.sync.dma_start(out=wh0[:], in_=wh_in[0:P, :])
            nc.sync.dma_start(out=wh1[:], in_=wh_in[P:H, :])
            wo1_t = cpool.tile([AF + 1, H], bf16)
            nc.sync.dma_start(out=wo1_t[:], in_=wo1_in[:])
            wo2_0 = cpool.tile([P, H], bf16)
            wo2_1 = cpool.tile([P, H], bf16)
            nc.sync.dma_start(out=wo2_0[:], in_=wo2_in[0:P, :])
            nc.sync.dma_start(out=wo2_1[:], in_=wo2_in[P:H, :])
            id_t = cpool.tile([P, P], bf16)
            nc.sync.dma_start(out=id_t[:], in_=id_in[:])
            na_all = cpool.tile([P, Kn, 2, P], bf16)
            idx_t = cpool.tile([P, n_groups, 2, IW], i16)
            nc.sync.dma_start(out=idx_t[:], in_=idx_in[:])
            mi_slab = cpool.tile([P, NCH, H], bf16)

            # ---- stage A: node_alpha^T (kept in bf16) ----
            for t in range(Kn):
                ps_na = [pp.tile([P, P], f32, tag=f"ps_s{i}", name=f"ps_na{i}")
                         for i in range(2)]
                for ct in range(SCT):
                    tr = sp.tile([P, H], f32, tag="tr")
                    nc.sync.dma_start(
                        out=tr[:],
                        in_=tree_in[(t * SCT + ct) * P:(t * SCT + ct + 1) * P, :])
                    st = sp.tile([P, P], f32, tag="st")
                    nc.sync.dma_start(out=st[:], in_=seltree_in[t, ct])
                    for s in range(2):
                        nc.tensor.matmul(
                            out=ps_na[s][:],
                            lhsT=tr[:, s * P:(s + 1) * P], rhs=st[:],
                            start=(ct == 0), stop=(ct == SCT - 1))
                for s in range(2):
                    nc.vector.tensor_copy(out=na_all[:, t, s, :],
                                          in_=ps_na[s][:])

            # ---- sweeps ----
            # chunk k of a group: class s = k//4, tile half = (k%4)//2
            def chunk_tile(g, k):
                return 2 * g + (k % 4) // 2

            TILE_CHUNKS = {0: (0, 1, 4, 5), 1: (2, 3, 6, 7)}

            for sw in range(DEPTH):
                last = sw == DEPTH - 1
                if last:
                    psG = [pacc.tile([P, H], f32, tag=f"psG{i}", name=f"psG{i}")
                           for i in range(2)]
                for g in range(n_groups):
                    base = 8 * g
                    msg = wp.tile([P, 8, H], bf16, tag="msg")
                    if sw == 0:
                        xe_g = wp.tile([KF, 8 * P], bf16, tag="xe")
                        nc.sync.dma_start(
                            out=xe_g[:],
                            in_=xe_in[:, base * P:(base + 8) * P])
                        for k in range(8):
                            ps_mi = pp.tile([P, H], f32, tag="ps_big",
                                            name="ps_mi")
                            nc.tensor.matmul(
                                out=ps_mi[:],
                                lhsT=xe_g[:, k * P:(k + 1) * P],
                                rhs=wi_t[:], start=True, stop=True)
                            nc.vector.tensor_copy(
                                out=mi_slab[:, base + k, :], in_=ps_mi[:])
                            nc.scalar.activation(
                                out=msg[:, k, :], in_=ps_mi[:], func=RELU)
                    else:
                        Up = U_tab[(sw + 1) % 2]
                        gt = [wp.tile([P, 4, H], bf16, tag=f"g{s}",
                                      name=f"g{s}") for s in range(2)]
                        if "nogather" not in abl:
                            for s in range(2):
                                nc.gpsimd.dma_gather(
                                    out_ap=gt[s][:], in_ap=Up[s][:],
                                    idxs_ap=idx_t[:, g, s, :],
                                    num_idxs=2 * CLASS_CAP,
                                    num_idxs_reg=2 * CLASS_CAP,
                                    elem_size=H, queue_num=0)
                        for s in range(2):
                            nc.vector.tensor_tensor(
                                out=msg[:, 4 * s:4 * s + 4, :].rearrange(
                                    "p a b -> p (a b)"),
                                in0=mi_slab[:, base + 4 * s:
                                            base + 4 * s + 4, :].rearrange(
                                    "p a b -> p (a b)"),
                                in1=gt[s][:].rearrange("p a b -> p (a b)"),
                                op=ADD)
                        nc.scalar.activation(
                            out=msg[:].rearrange("p a b -> p (a b)"),
                            in_=msg[:].rearrange("p a b -> p (a b)"),
                            func=RELU)
                    selg = wp.tile([P, 8, P], bf16, tag="selg")
                    nc.sync.dma_start(out=selg[:], in_=sel_in[g])
                    for half in range(2):
                        t = 2 * g + half
                        psS = [pp.tile([P, P], f32, tag=f"ps_s{i}",
                                       name=f"psS{i}") for i in range(2)]
                        for ci, k in enumerate(TILE_CHUNKS[half]):
                            for s in range(2):
                                nc.tensor.matmul(
                                    out=psS[s][:],
                                    lhsT=msg[:, k, s * P:(s + 1) * P],
                                    rhs=selg[:, k, :],
                                    start=(ci == 0), stop=False)
                        # += node_alpha via identity matmul (closes psum)
                        for s in range(2):
                            nc.tensor.matmul(
                                out=psS[s][:], lhsT=id_t[:],
                                rhs=na_all[:, t, s, :],
                                start=False, stop=True)
                        TT = [sp.tile([P, P], bf16, tag=f"TT{i}",
                                      name=f"TT{i}") for i in range(2)]
                        for s in range(2):
                            nc.vector.tensor_copy(out=TT[s][:], in_=psS[s][:])
                        if not last:
                            psU = pp.tile([P, H], f32, tag="ps_big",
                                          name="psU")
                            nc.tensor.matmul(out=psU[:], lhsT=TT[0][:],
                                             rhs=wh0[:], start=True,
                                             stop=False)
                            nc.tensor.matmul(out=psU[:], lhsT=TT[1][:],
                                             rhs=wh1[:], start=False,
                                             stop=True)
                            u_sb = sp.tile([P, H], bf16, tag="u_sb")
                            nc.vector.tensor_copy(out=u_sb[:], in_=psU[:])
                            agt = ag_in[0] if t < Kh else ag_in[1]
                            tl = t if t < Kh else t - Kh
                            nc.sync.dma_start(
                                out=agt[tl * P:(tl + 1) * P, :], in_=u_sb[:])
                            if t == Kh - 1 and "noag" not in abl:
                                nc.gpsimd.collective_compute(
                                    "AllGather", mybir.AluOpType.bypass,
                                    replica_groups=[list(range(N_CORES))],
                                    ins=[ag_in[0][:].opt()],
                                    outs=[U_tab[sw % 2][0][:].opt()])
                        else:
                            xn_t = sp.tile([AF + 1, P], bf16, tag="xn_t")
                            nc.sync.dma_start(
                                out=xn_t[:],
                                in_=xn_in[:, t * P:(t + 1) * P])
                            psH = pp.tile([P, H], f32, tag="ps_big",
                                          name="psH")
                            nc.tensor.matmul(out=psH[:], lhsT=xn_t[:],
                                             rhs=wo1_t[:], start=True,
                                             stop=False)
                            nc.tensor.matmul(out=psH[:], lhsT=TT[0][:],
                                             rhs=wo2_0[:], start=False,
                                             stop=False)
                            nc.tensor.matmul(out=psH[:], lhsT=TT[1][:],
                                             rhs=wo2_1[:], start=False,
                                             stop=True)
                            h_sb = sp.tile([P, H], f32, tag="h_sb")
                            nc.scalar.activation(out=h_sb[:], in_=psH[:],
                                                 func=RELU)
                            pw_t = sp.tile([P, GCAP], f32, tag="pw_t")
                            nc.sync.dma_start(out=pw_t[:], in_=poolw_in[t])
                            for s in range(2):
                                nc.tensor.matmul(
                                    out=psG[s][:],
                                    lhsT=pw_t[:, s * P:(s + 1) * P],
                                    rhs=h_sb[:],
                                    start=(t == 0), stop=(t == Kn - 1))
                if not last:
                    if "noag" not in abl:
                        nc.gpsimd.collective_compute(
                            "AllGather", mybir.AluOpType.bypass,
                            replica_groups=[list(range(N_CORES))],
                            ins=[ag_in[1][:].opt()],
                            outs=[U_tab[sw % 2][1][:].opt()])
                else:
                    for s in range(2):
                        og = sp.tile([P, H], f32, tag="og")
                        nc.vector.tensor_copy(out=og[:], in_=psG[s][:])
                        nc.sync.dma_start(
                            out=out_t[s * P:(s + 1) * P, :], in_=og[:])

    nc.compile()
    return nc


# ======================================================================
# Entry point
# ======================================================================

_last_results = None


def kernel(**inputs):
    from concourse.bass_utils import run_bass_kernel_spmd

    per_core, meta, glo_ghi = _prep(inputs)
    nc = _build(meta)
    in_maps = [{k: v for k, v in pc.items()} for pc in per_core]
    res = run_bass_kernel_spmd(nc, in_maps, core_ids=list(range(N_CORES)))
    global _last_results
    _last_results = res

    G = meta["n_graphs"]
    out = np.zeros((G, H), F32)
    for c in range(N_CORES):
        glo, ghi = glo_ghi[c]
        out[glo:ghi] = res.results[c]["out"][: ghi - glo]
    return out


# revision 10
# speedup vs baseline: 1.6715x; 1.0053x over previous
"""DGLJTMPN message-passing network on 8 Trainium2 NeuronCores (Bass/Tile).

Algorithm (mathematically identical to the reference):
  The loopy-BP line-graph aggregation  accum = segment_sum(msg[lg_src], lg_dst)
  is rewritten with node-level sums:  accum[e] = S[src[e]] - (backtracking
  partners), where S = segment_sum(msg, edge_dst).  The missing/backtracking
  pairs (the complement of the given lg list w.r.t. the full line graph) are
  folded into extra host-built one-hot "virtual columns", so each edge reads
  exactly one row of U = (S + node_alpha) @ W_h per iteration.

Sharding: nodes/edges/graphs are split into 8 contiguous graph-aligned
ranges; edges live on the core owning their dst node, so S/U shards are
disjoint.  Per iteration each core computes its U shard, an AllGather
replicates U (in two int16-addressable half tables), and a runtime-indexed
dma_gather fetches U[src[e]].  Each tile's edge slab is split into two
256-slot class regions by src half-table, so every edge is gathered exactly
once (no zero-row double gathers).  msg_input stays resident in SBUF across
all sweeps.  Scatter-adds are one-hot matmuls on the tensor engine.
"""

import numpy as np
import ml_dtypes

P = 128
SC = 4            # edge chunks per node tile
EDGE_CAP = SC * P
CLASS_CAP = EDGE_CAP // 2   # per-tile cap on edges per src-table class
H = 256
N_CORES = 8
DEPTH = 4
GCAP = 2 * P      # max graphs per core

F32 = np.float32
BF16 = ml_dtypes.bfloat16


# ======================================================================
# Host preprocessing
# ======================================================================

def _full_line_graph_keys(src, dst, E, N):
    indeg = np.bincount(dst, minlength=N)
    idx_sorted = np.argsort(dst, kind="stable")
    ptr = np.concatenate([[0], np.cumsum(indeg)]).astype(np.int64)
    counts = indeg[src]
    total = int(counts.sum())
    lg_dst = np.repeat(np.arange(E, dtype=np.int64), counts)
    cum = np.cumsum(counts) - counts
    within = np.arange(total) - np.repeat(cum, counts)
    lg_src = idx_sorted[np.repeat(ptr[src], counts) + within]
    return lg_src * E + lg_dst


def _prep(inputs, n_cores=N_CORES):
    x_nodes = np.asarray(inputs["x_nodes"], F32)
    x_edges = np.asarray(inputs["x_edges"], F32)
    tree_m = np.asarray(inputs["tree_m"], F32)
    W_i = np.asarray(inputs["W_i"], F32)
    W_h = np.asarray(inputs["W_h"], F32)
    W_o = np.asarray(inputs["W_o"], F32)
    b_o = np.asarray(inputs["b_o"], F32)
    src = np.asarray(inputs["edge_src"], np.int64)
    dst = np.asarray(inputs["edge_dst"], np.int64)
    lg_src = np.asarray(inputs["lg_src"], np.int64)
    lg_dst = np.asarray(inputs["lg_dst"], np.int64)
    tgt_nodes = np.asarray(inputs["tgt_nodes"], np.int64)
    tree_eid = np.asarray(inputs["tree_eid"], np.int64)
    graph_ids = np.asarray(inputs["graph_ids"], np.int64)
    n_graphs = int(inputs["n_graphs"])

    N = x_nodes.shape[0]
    E = x_edges.shape[0]
    AF = x_nodes.shape[1]
    KF = AF + x_edges.shape[1]

    # corrections: full-line-graph pairs missing from the given lg list
    full_keys = _full_line_graph_keys(src, dst, E, N)
    given_keys = lg_src * E + lg_dst
    missing = np.setdiff1d(full_keys, given_keys)
    assert np.setdiff1d(given_keys, full_keys).size == 0
    miss_e1 = (missing // E).astype(np.int64)
    miss_e2 = (missing % E).astype(np.int64)
    assert np.all(dst[miss_e1] == src[miss_e2])
    order = np.argsort(miss_e2, kind="stable")
    miss_e1, miss_e2 = miss_e1[order], miss_e2[order]
    corr_e2, corr_start = np.unique(miss_e2, return_index=True)
    corr_partners = {}
    for i, e2 in enumerate(corr_e2):
        lo = corr_start[i]
        hi = corr_start[i + 1] if i + 1 < len(corr_e2) else len(miss_e2)
        corr_partners[int(e2)] = miss_e1[lo:hi]
    virt_nodes = src[corr_e2] if len(corr_e2) else np.array([], np.int64)
    vdemand = np.bincount(virt_nodes, minlength=N)
    corr_by_node = {}
    for e2 in corr_e2:
        corr_by_node.setdefault(int(src[e2]), []).append(int(e2))

    # graph-aligned node cuts
    gcnt = np.bincount(graph_ids, minlength=n_graphs)
    gcum = np.concatenate([[0], np.cumsum(gcnt)])
    cuts = [0]
    for c in range(1, n_cores):
        g = int(np.argmin(np.abs(gcum - c * N / n_cores)))
        cuts.append(int(gcum[g]))
    cuts.append(N)
    cuts = np.asarray(cuts, np.int64)
    assert np.all(np.diff(cuts) > 0)

    indeg = np.bincount(dst, minlength=N)
    assert indeg.max() <= CLASS_CAP
    edges_by_dst = np.argsort(dst, kind="stable")
    eptr = np.concatenate([[0], np.cumsum(indeg)]).astype(np.int64)
    tdeg = np.bincount(tgt_nodes, minlength=N)
    tpairs_by_tgt = np.argsort(tgt_nodes, kind="stable")
    tptr = np.concatenate([[0], np.cumsum(tdeg)]).astype(np.int64)

    # ---- tile packing with per-class edge caps ----
    # class of edge e = half-table of src[e]'s U row, which depends on the
    # packing itself (tile index < Kh -> table 0).  Pack with an estimated
    # class map and a safety margin on the per-class caps, then accept any
    # packing whose TRUE per-tile class counts fit within CLASS_CAP.
    # Iteration 0 packs without class caps to seed the estimates.
    ecls = np.zeros(E, np.int8)
    margin = 16
    for _it in range(12):
        class_capped = _it > 0
        ccap = CLASS_CAP - margin
        per_core_tiles = []
        for c in range(n_cores):
            nlo, nhi = int(cuts[c]), int(cuts[c + 1])
            tiles, cur, cur_slots = [], [], 0
            cur_c = [0, 0]
            cap = P - 1 if c == 0 else P
            for n in range(nlo, nhi):
                ns = 1 + int(vdemand[n])
                ee = edges_by_dst[eptr[n]:eptr[n + 1]]
                n0 = int(np.count_nonzero(ecls[ee] == 0))
                n1 = len(ee) - n0
                if class_capped:
                    full = (cur_c[0] + n0 > ccap or cur_c[1] + n1 > ccap)
                else:
                    full = cur_c[0] + cur_c[1] + n0 + n1 > EDGE_CAP
                if cur and (cur_slots + ns > cap or full):
                    tiles.append(cur)
                    cur, cur_slots, cur_c = [], 0, [0, 0]
                cur.append(n)
                cur_slots += ns
                cur_c[0] += n0
                cur_c[1] += n1
            if cur:
                tiles.append(cur)
            per_core_tiles.append(tiles)

        Kn = max(len(t) for t in per_core_tiles)
        Kn = -(-Kn // 4) * 4
        Kh = Kn // 2
        CORE_ROWS = P * Kn
        R = CORE_ROWS * n_cores
        RT = R // 2
        HROWS = P * Kh

        def tab_row(c, t, j):
            if t < Kh:
                return 0, HROWS * c + P * t + j
            return 1, HROWS * c + P * (t - Kh) + j

        node_row = np.full(N, -1, np.int64)
        virt_slot = {}
        node_tj = {}
        for c in range(n_cores):
            for t, tile in enumerate(per_core_tiles[c]):
                j = 0
                for n in tile:
                    tb, rw = tab_row(c, t, j)
                    node_row[n] = tb * RT + rw
                    node_tj[n] = (t, j)
                    j += 1
                    for e2 in corr_by_node.get(n, []):
                        tb, rw = tab_row(c, t, j)
                        virt_slot[e2] = tb * RT + rw
                        j += 1
                assert j <= (P - 1 if c == 0 else P)
        assert np.all(node_row >= 0)
        edge_row = node_row[src].copy()
        for e2, row in virt_slot.items():
            edge_row[e2] = row

        true_cls = (edge_row >= RT).astype(np.int8)
        if class_capped:
            ok = True
            for c in range(n_cores):
                for tile in per_core_tiles[c]:
                    tc = [0, 0]
                    for n in tile:
                        for e in edges_by_dst[eptr[n]:eptr[n + 1]]:
                            tc[true_cls[e]] += 1
                    if tc[0] > CLASS_CAP or tc[1] > CLASS_CAP:
                        ok = False
            if ok:
                ecls = true_cls
                break
            margin = min(margin + 16, 96)
        ecls = true_cls
    else:
        raise RuntimeError("edge-class packing did not converge")
    assert RT <= 32767, RT

    n_groups = Kh               # 2 tiles per group
    E_slab = Kn * EDGE_CAP
    GROUP = 2 * EDGE_CAP        # slots per group

    SCT = 1
    for c in range(n_cores):
        for tile in per_core_tiles[c]:
            SCT = max(SCT, -(-int(sum(tdeg[n] for n in tile)) // P))
    T_slab = Kn * SCT * P

    meta = dict(N=N, E=E, AF=AF, KF=KF, Kn=Kn, Kh=Kh, SCT=SCT,
                CORE_ROWS=CORE_ROWS, R=R, RT=RT, E_slab=E_slab,
                n_groups=n_groups, T_slab=T_slab, n_cores=n_cores,
                n_graphs=n_graphs, n_corr=len(corr_e2))

    # zero rows (core 0 caps slots at P-1, so slot P-1 of tile 0 / tile Kh
    # is always free there): local row 127 in each table.
    z_local = [P - 1, P - 1]

    def wrap(idx):
        # idx [512] -> [128, 32] with idx j at (partition j%16, col j//16)
        w = idx.reshape(-1, 16)            # [col, lane]
        w = np.ascontiguousarray(w.T)      # [16, col]
        w = np.tile(w, (P // 16, 1))
        return w.astype(np.int16)

    glo_ghi = []
    per_core = []
    for c in range(n_cores):
        nlo, nhi = int(cuts[c]), int(cuts[c + 1])
        tiles = per_core_tiles[c]
        glo = int(graph_ids[nlo])
        ghi = int(graph_ids[nhi - 1]) + 1
        assert ghi - glo <= GCAP
        glo_ghi.append((glo, ghi))

        sel = np.zeros((n_groups, 8, P, P), F32)
        seltree = np.zeros((Kn, SCT, P, P), F32)
        tree_slab = np.zeros((T_slab, H), F32)
        xe_catT = np.zeros((KF, E_slab), F32)
        xnodesT = np.zeros((AF + 1, CORE_ROWS), F32)
        xnodesT[AF, :] = 1.0
        poolw = np.zeros((Kn, P, GCAP), F32)
        idx_all = np.zeros((n_groups, 2, P, GROUP // 2 // 16), np.int16)

        for t, tile in enumerate(tiles):
            g, half = t // 2, t % 2
            # per-class edge placement: class s region of tile t =
            # group chunks {4s + 2*half, 4s + 2*half + 1}
            kcnt = [0, 0]           # edges placed per class
            pos_of_edge = {}
            rows_cls = [[], []]
            for n in tile:
                j = node_tj[n][1]
                xnodesT[:AF, P * t + j] = x_nodes[n]
                gg = int(graph_ids[n])
                poolw[t, j, gg - glo] = 1.0 / max(int(gcnt[gg]), 1)
                for e in edges_by_dst[eptr[n]:eptr[n + 1]]:
                    s = int(ecls[e])
                    k_in = kcnt[s]
                    kcnt[s] += 1
                    chunk = 4 * s + 2 * half + k_in // P
                    i = k_in % P
                    slab_pos = (8 * g + chunk) * P + i
                    pos_of_edge[int(e)] = (chunk, i)
                    sel[g, chunk, i, j] = 1.0
                    row = int(edge_row[e]) - s * RT
                    rows_cls[s].append(row)
                    xe_catT[:AF, slab_pos] = x_nodes[src[e]]
                    xe_catT[AF:, slab_pos] = x_edges[e]
            assert kcnt[0] <= CLASS_CAP and kcnt[1] <= CLASS_CAP
            for s in range(2):
                lst = rows_cls[s] + [z_local[s]] * (CLASS_CAP - kcnt[s])
                arr = np.asarray(lst, np.int64)
                assert arr.min() >= 0 and arr.max() < RT
                # tile half occupies positions half*CLASS_CAP .. within the
                # group's class-s gather (512 idxs spanning both tiles)
                idx_all[g, s, :, half * (CLASS_CAP // 16):
                       (half + 1) * (CLASS_CAP // 16)] = wrap(arr)
            # virtual columns
            for n in tile:
                jn = node_tj[n][1]
                for vi, e2 in enumerate(corr_by_node.get(n, [])):
                    jv = jn + 1 + vi
                    partners = set(corr_partners[e2].tolist())
                    for e in edges_by_dst[eptr[n]:eptr[n + 1]]:
                        if int(e) in partners:
                            continue
                        ck, ii = pos_of_edge[int(e)]
                        sel[g, ck, ii, jv] = 1.0
            # tree pairs
            kt = 0
            for n in tile:
                j = node_tj[n][1]
                nvirt = len(corr_by_node.get(n, []))
                for pidx in tpairs_by_tgt[tptr[n]:tptr[n + 1]]:
                    tree_slab[SCT * P * t + kt] = tree_m[tree_eid[pidx]]
                    seltree[t, kt // P, kt % P, j] = 1.0
                    for vi in range(nvirt):
                        seltree[t, kt // P, kt % P, j + 1 + vi] = 1.0
                    kt += 1
            assert kt <= SCT * P

        # sel regrouped for DMA: [n_groups, 128(edge), 8, 128(slot)]
        selg = np.ascontiguousarray(np.transpose(sel, (0, 2, 1, 3)))
        ident = np.eye(P, dtype=F32)
        per_core.append(dict(
            xe_catT=xe_catT.astype(BF16),
            sel=selg.astype(BF16),
            seltree=seltree,
            tree_slab=tree_slab,
            xnodesT=xnodesT.astype(BF16),
            poolw=poolw,
            idxs=np.ascontiguousarray(
                np.transpose(idx_all, (2, 0, 1, 3))),   # [128, g, 2, 32]
            ident=ident.astype(BF16),
            wi=W_i.astype(BF16),
            wh=W_h.astype(BF16),
            wo1=np.ascontiguousarray(
                np.concatenate([W_o[:AF], b_o[None, :]], 0).astype(BF16)),
            wo2=np.ascontiguousarray(W_o[AF:].astype(BF16)),
        ))

    return per_core, meta, glo_ghi


# ======================================================================
# Bass program
# ======================================================================

def _build(meta):
    import os
    abl = set(os.environ.get("KERNEL_ABL", "").split(","))
    import concourse.bacc as bacc
    import concourse.tile as tile
    from concourse import mybir

    Kn, Kh, SCT = meta["Kn"], meta["Kh"], meta["SCT"]
    CORE_ROWS, R, RT = meta["CORE_ROWS"], meta["R"], meta["RT"]
    E_slab, n_groups, T_slab = meta["E_slab"], meta["n_groups"], meta["T_slab"]
    KF, AF = meta["KF"], meta["AF"]
    HROWS = P * Kh
    NCH = E_slab // P           # total slab chunks ( = 8 * n_groups )
    IW = CLASS_CAP * 2 // 16    # idx cols per (group, class) = 32

    f32, bf16, i16 = mybir.dt.float32, mybir.dt.bfloat16, mybir.dt.int16
    RELU = mybir.ActivationFunctionType.Relu
    ADD = mybir.AluOpType.add

    nc = bacc.Bacc("TRN2", target_bir_lowering=False, num_devices=N_CORES)

    # kernel I/O
    xe_in = nc.dram_tensor("xe_catT", [KF, E_slab], bf16, kind="ExternalInput")
    sel_in = nc.dram_tensor("sel", [n_groups, P, 8, P], bf16,
                            kind="ExternalInput")
    seltree_in = nc.dram_tensor("seltree", [Kn, SCT, P, P], f32,
                                kind="ExternalInput")
    tree_in = nc.dram_tensor("tree_slab", [T_slab, H], f32,
                             kind="ExternalInput")
    xn_in = nc.dram_tensor("xnodesT", [AF + 1, CORE_ROWS], bf16,
                           kind="ExternalInput")
    poolw_in = nc.dram_tensor("poolw", [Kn, P, GCAP], f32,
                              kind="ExternalInput")
    idx_in = nc.dram_tensor("idxs", [P, n_groups, 2, IW], i16,
                            kind="ExternalInput")
    id_in = nc.dram_tensor("ident", [P, P], bf16, kind="ExternalInput")
    wi_in = nc.dram_tensor("wi", [KF, H], bf16, kind="ExternalInput")
    wh_in = nc.dram_tensor("wh", [H, H], bf16, kind="ExternalInput")
    wo1_in = nc.dram_tensor("wo1", [AF + 1, H], bf16, kind="ExternalInput")
    wo2_in = nc.dram_tensor("wo2", [H, H], bf16, kind="ExternalInput")
    out_t = nc.dram_tensor("out", [GCAP, H], f32, kind="ExternalOutput")

    # internal DRAM
    ag_in = [nc.dram_tensor(f"ag_in_{h}", [HROWS, H], bf16) for h in range(2)]
    U_tab = [[nc.dram_tensor(f"U_{h}{i}", [RT, H], bf16, addr_space="Shared")
              for h in range(2)] for i in range(2)]

    with tile.TileContext(nc) as tc:
        RUNAHEAD = 8
        with tc.tile_pool(name="const", bufs=1) as cpool, \
             tc.tile_pool(name="work", bufs=2) as wp, \
             tc.tile_pool(name="g0pool", bufs=RUNAHEAD + 1) as g0p, \
             tc.tile_pool(name="small", bufs=3) as sp, \
             tc.tile_pool(name="psum", bufs=2, space="PSUM") as pp, \
             tc.tile_pool(name="psum_acc", bufs=1, space="PSUM") as pacc:

            # ---- resident constants ----
            wi_t = cpool.tile([KF, H], bf16)
            nc.sync.dma_start(out=wi_t[:], in_=wi_in[:])
            wh0 = cpool.tile([P, H], bf16)
            wh1 = cpool.tile([P, H], bf16)
            nc.sync.dma_start(out=wh0[:], in_=wh_in[0:P, :])
            nc.sync.dma_start(out=wh1[:], in_=wh_in[P:H, :])
            wo1_t = cpool.tile([AF + 1, H], bf16)
            nc.sync.dma_start(out=wo1_t[:], in_=wo1_in[:])
            wo2_0 = cpool.tile([P, H], bf16)
            wo2_1 = cpool.tile([P, H], bf16)
            nc.sync.dma_start(out=wo2_0[:], in_=wo2_in[0:P, :])
            nc.sync.dma_start(out=wo2_1[:], in_=wo2_in[P:H, :])
            id_t = cpool.tile([P, P], bf16)
            nc.sync.dma_start(out=id_t[:], in_=id_in[:])
            na_all = cpool.tile([P, Kn, 2, P], bf16)
            idx_t = cpool.tile([P, n_groups, 2, IW], i16)
            nc.sync.dma_start(out=idx_t[:], in_=idx_in[:])
            mi_slab = cpool.tile([P, NCH, H], bf16)

            # ---- stage A: node_alpha^T (kept in bf16) ----
            for t in range(Kn):
                ps_na = [pp.tile([P, P], f32, tag=f"ps_s{i}", name=f"ps_na{i}")
                         for i in range(2)]
                for ct in range(SCT):
                    tr = sp.tile([P, H], f32, tag="tr")
                    nc.sync.dma_start(
                        out=tr[:],
                        in_=tree_in[(t * SCT + ct) * P:(t * SCT + ct + 1) * P, :])
                    st = sp.tile([P, P], f32, tag="st")
                    nc.sync.dma_start(out=st[:], in_=seltree_in[t, ct])
                    for s in range(2):
                        nc.tensor.matmul(
                            out=ps_na[s][:],
                            lhsT=tr[:, s * P:(s + 1) * P], rhs=st[:],
                            start=(ct == 0), stop=(ct == SCT - 1))
                for s in range(2):
                    nc.vector.tensor_copy(out=na_all[:, t, s, :],
                                          in_=ps_na[s][:])

            # ---- sweeps ----
            # chunk k of a group: class s = k//4, tile half = (k%4)//2
            def chunk_tile(g, k):
                return 2 * g + (k % 4) // 2

            TILE_CHUNKS = {0: (0, 1, 4, 5), 1: (2, 3, 6, 7)}

            for sw in range(DEPTH):
                last = sw == DEPTH - 1
                if last:
                    psG = [pacc.tile([P, H], f32, tag=f"psG{i}", name=f"psG{i}")
                           for i in range(2)]

                # class-0 gathers run ahead of class-1 gathers so the gpsimd
                # queue has stall-free work while AG of table 1 completes
                g0_tiles = {}

                def issue_g0(g, sw=sw):
                    t = g0p.tile([P, 4, H], bf16, tag="g0")
                    if "nogather" not in abl:
                        nc.gpsimd.dma_gather(
                            out_ap=t[:], in_ap=U_tab[(sw + 1) % 2][0][:],
                            idxs_ap=idx_t[:, g, 0, :],
                            num_idxs=2 * CLASS_CAP,
                            num_idxs_reg=2 * CLASS_CAP,
                            elem_size=H, queue_num=0)
                    g0_tiles[g] = t

                if sw > 0:
                    for g in range(min(RUNAHEAD, n_groups)):
                        issue_g0(g)

                for g in range(n_groups):
                    base = 8 * g
                    msg = wp.tile([P, 8, H], bf16, tag="msg")
                    if sw == 0:
                        xe_g = wp.tile([KF, 8 * P], bf16, tag="xe")
                        nc.sync.dma_start(
                            out=xe_g[:],
                            in_=xe_in[:, base * P:(base + 8) * P])
                        for k in range(8):
                            ps_mi = pp.tile([P, H], f32, tag="ps_big",
                                            name="ps_mi")
                            nc.tensor.matmul(
                                out=ps_mi[:],
                                lhsT=xe_g[:, k * P:(k + 1) * P],
                                rhs=wi_t[:], start=True, stop=True)
                            nc.vector.tensor_copy(
                                out=mi_slab[:, base + k, :], in_=ps_mi[:])
                            nc.scalar.activation(
                                out=msg[:, k, :], in_=ps_mi[:], func=RELU)
                    else:
                        if g + RUNAHEAD < n_groups:
                            issue_g0(g + RUNAHEAD)
                        g1 = wp.tile([P, 4, H], bf16, tag="g1", name="g1")
                        if "nogather" not in abl:
                            nc.gpsimd.dma_gather(
                                out_ap=g1[:], in_ap=U_tab[(sw + 1) % 2][1][:],
                                idxs_ap=idx_t[:, g, 1, :],
                                num_idxs=2 * CLASS_CAP,
                                num_idxs_reg=2 * CLASS_CAP,
                                elem_size=H, queue_num=0)
                        gt = [g0_tiles.pop(g), g1]
                        for s in range(2):
                            nc.vector.tensor_tensor(
                                out=msg[:, 4 * s:4 * s + 4, :].rearrange(
                                    "p a b -> p (a b)"),
                                in0=mi_slab[:, base + 4 * s:
                                            base + 4 * s + 4, :].rearrange(
                                    "p a b -> p (a b)"),
                                in1=gt[s][:].rearrange("p a b -> p (a b)"),
                                op=ADD)
                        nc.scalar.activation(
                            out=msg[:].rearrange("p a b -> p (a b)"),
                            in_=msg[:].rearrange("p a b -> p (a b)"),
                            func=RELU)
                    selg = wp.tile([P, 8, P], bf16, tag="selg")
                    nc.sync.dma_start(out=selg[:], in_=sel_in[g])
                    for half in range(2):
                        t = 2 * g + half
                        psS = [pp.tile([P, P], f32, tag=f"ps_s{i}",
                                       name=f"psS{i}") for i in range(2)]
                        for ci, k in enumerate(TILE_CHUNKS[half]):
                            for s in range(2):
                                nc.tensor.matmul(
                                    out=psS[s][:],
                                    lhsT=msg[:, k, s * P:(s + 1) * P],
                                    rhs=selg[:, k, :],
                                    start=(ci == 0), stop=False)
                        # += node_alpha via identity matmul (closes psum)
                        for s in range(2):
                            nc.tensor.matmul(
                                out=psS[s][:], lhsT=id_t[:],
                                rhs=na_all[:, t, s, :],
                                start=False, stop=True)
                        TT = [sp.tile([P, P], bf16, tag=f"TT{i}",
                                      name=f"TT{i}") for i in range(2)]
                        for s in range(2):
                            nc.vector.tensor_copy(out=TT[s][:], in_=psS[s][:])
                        if not last:
                            psU = pp.tile([P, H], f32, tag="ps_big",
                                          name="psU")
                            nc.tensor.matmul(out=psU[:], lhsT=TT[0][:],
                                             rhs=wh0[:], start=True,
                                             stop=False)
                            nc.tensor.matmul(out=psU[:], lhsT=TT[1][:],
                                             rhs=wh1[:], start=False,
                                             stop=True)
                            u_sb = sp.tile([P, H], bf16, tag="u_sb")
                            nc.vector.tensor_copy(out=u_sb[:], in_=psU[:])
                            agt = ag_in[0] if t < Kh else ag_in[1]
                            tl = t if t < Kh else t - Kh
                            nc.sync.dma_start(
                                out=agt[tl * P:(tl + 1) * P, :], in_=u_sb[:])
                            if t == Kh - 1 and "noag" not in abl:
                                nc.gpsimd.collective_compute(
                                    "AllGather", mybir.AluOpType.bypass,
                                    replica_groups=[list(range(N_CORES))],
                                    ins=[ag_in[0][:].opt()],
                                    outs=[U_tab[sw % 2][0][:].opt()])
                        else:
                            xn_t = sp.tile([AF + 1, P], bf16, tag="xn_t")
                            nc.sync.dma_start(
                                out=xn_t[:],
                                in_=xn_in[:, t * P:(t + 1) * P])
                            psH = pp.tile([P, H], f32, tag="ps_big",
                                          name="psH")
                            nc.tensor.matmul(out=psH[:], lhsT=xn_t[:],
                                             rhs=wo1_t[:], start=True,
                                             stop=False)
                            nc.tensor.matmul(out=psH[:], lhsT=TT[0][:],
                                             rhs=wo2_0[:], start=False,
                                             stop=False)
                            nc.tensor.matmul(out=psH[:], lhsT=TT[1][:],
                                             rhs=wo2_1[:], start=False,
                                             stop=True)
                            h_sb = sp.tile([P, H], f32, tag="h_sb")
                            nc.scalar.activation(out=h_sb[:], in_=psH[:],
                                                 func=RELU)
                            pw_t = sp.tile([P, GCAP], f32, tag="pw_t")
                            nc.sync.dma_start(out=pw_t[:], in_=poolw_in[t])
                            for s in range(2):
                                nc.tensor.matmul(
                                    out=psG[s][:],
                                    lhsT=pw_t[:, s * P:(s + 1) * P],
                                    rhs=h_sb[:],
                                    start=(t == 0), stop=(t == Kn - 1))
                if not last:
                    if "noag" not in abl:
                        nc.gpsimd.collective_compute(
                            "AllGather", mybir.AluOpType.bypass,
                            replica_groups=[list(range(N_CORES))],
                            ins=[ag_in[1][:].opt()],
                            outs=[U_tab[sw % 2][1][:].opt()])
                else:
                    for s in range(2):
                        og = sp.tile([P, H], f32, tag="og")
                        nc.vector.tensor_copy(out=og[:], in_=psG[s][:])
                        nc.sync.dma_start(
                            out=out_t[s * P:(s + 1) * P, :], in_=og[:])

    nc.compile()
    return nc


# ======================================================================
# Entry point
# ======================================================================

_last_results = None


def kernel(**inputs):
    from concourse.bass_utils import run_bass_kernel_spmd

    per_core, meta, glo_ghi = _prep(inputs)
    nc = _build(meta)
    in_maps = [{k: v for k, v in pc.items()} for pc in per_core]
    res = run_bass_kernel_spmd(nc, in_maps, core_ids=list(range(N_CORES)))
    global _last_results
    _last_results = res

    G = meta["n_graphs"]
    out = np.zeros((G, H), F32)
    for c in range(N_CORES):
        glo, ghi = glo_ghi[c]
        out[glo:ghi] = res.results[c]["out"][: ghi - glo]
    return out
